# revision 42
# baseline (speedup 1.0000x reference)
"""Trainium2 Bass kernel for fused multi-head attention (16 heads, d_model=1024).

Computes, for x[2, 2048, 1024], w_qkv[3072, 1024], b_qkv[3072]:
    qkv = x @ w_qkv.T + b_qkv
    q, k, v per head (head-interleaved qkv layout)
    out = softmax(q k^T / sqrt(64)) v     reshaped head-major to [2, 2048, 1024]

Sharding: 8 cores = 2 batches x 4 head-groups. Core (b, g) handles batch b,
heads [4g, 4g+4). Everything is local per core; the host slices inputs and
concatenates outputs (the output layout is head-major, so each core's result
is a contiguous slab of the full output).

Per-core dataflow (all on one NeuronCore; matmul operands in fp16, fp32
accumulation in PSUM — measured ~4e-4 max relative error vs the fp32
reference):
  - host supplies x[b].T (xt, [1024, 2048]) and the core's 768 w_qkv rows,
    transposed and reordered pairwise ([Q(hA)|Q(hB)|K(hA)|K(hB)|V(hA)|V(hB)]
    per head pair), plus the matching bias.
  - projection: psum[feat, s] = wt.T @ xt accumulated over 8 c-tiles; bias
    added during the PSUM->SBUF copy. Q/K stay feature-major ([d, s], the
    layout the scores matmul wants); V is transposed on the PE to [s, d] and
    a ones column is appended (column 64).
  - attention per head: S^T[k, q] = K_t.T @ Q_t (PE, contraction over d=64),
    E = exp(S/8) (ACT, PSUM->SBUF), O[65, q] = [V|1]^T @ E accumulated over
    k-tiles (PE). Row 64 of O is the softmax denominator. O tiles are
    PE-transposed to [q, 65] and each row is scaled by 1/O[q, 64] (DVE).
"""

import os
import sys

import numpy as np

if "/opt/trn_rl_repo" not in sys.path:
    sys.path.insert(0, "/opt/trn_rl_repo")

B = 2
S = 2048
D_MODEL = 1024
NUM_HEADS = 16
HD = 64
N_CORES = 8

P = 128
CT = D_MODEL // P  # 8 contraction tiles for the projection
KT = S // P  # 16 key tiles
SB = 512  # projection s-block (matmul N)
QB = 1024  # attention q-block (one exp instruction)
NQB = S // QB  # 2
PAIRS = 2  # head pairs per core
HPC = 2 * PAIRS  # heads per core
FPC = HPC * 3 * HD  # 768 w rows per core

MM_DTYPE = os.environ.get("BASS_MM_DTYPE", "f16")
S_PSUM_F16 = os.environ.get("BASS_SPSUM", "f32") == "f16"

_CACHE = {}


def _build_program():
    import concourse.bacc as bacc
    import concourse.mybir as mybir
    import concourse.tile as tile
    from concourse.masks import make_identity

    f32 = mybir.dt.float32
    f32r = mybir.dt.float32r
    bf16 = mybir.dt.bfloat16
    f16 = mybir.dt.float16
    mmdt = {"bf16": bf16, "f16": f16, "f32r": f32r}[MM_DTYPE]
    EXP = mybir.ActivationFunctionType.Exp

    nc = bacc.Bacc("TRN2")

    xt_d = nc.dram_tensor("xt", [S // SB, D_MODEL, SB], mmdt, kind="ExternalInput")
    wt_d = nc.dram_tensor("wt", [D_MODEL, FPC], mmdt, kind="ExternalInput")
    bias_d = nc.dram_tensor("bias", [PAIRS * 3, P], f32, kind="ExternalInput")
    out_d = nc.dram_tensor("out", [HPC, S, HD], f32, kind="ExternalOutput")

    with tile.TileContext(nc) as tc:
        from contextlib import ExitStack

        with ExitStack() as ctx:
            const = ctx.enter_context(tc.tile_pool(name="const", bufs=1))
            qkp = ctx.enter_context(tc.tile_pool(name="qkp", bufs=1))
            vop = ctx.enter_context(tc.tile_pool(name="vop", bufs=1))
            vtp = ctx.enter_context(tc.tile_pool(name="vtp", bufs=2))
            etp = ctx.enter_context(tc.tile_pool(name="etp", bufs=8))
            osbp = ctx.enter_context(tc.tile_pool(name="osbp", bufs=2))
            ofinp = ctx.enter_context(tc.tile_pool(name="ofinp", bufs=2))
            rcp = ctx.enter_context(tc.tile_pool(name="rcp", bufs=8))
            # PSUM: big = 2-bank tiles (S scores + projection), small = 1-bank
            # (O accumulators + V transposes), tp = 1-bank (output transposes).
            bigp = ctx.enter_context(tc.tile_pool(name="bigp", bufs=2, space="PSUM"))
            smallp = ctx.enter_context(tc.tile_pool(name="smallp", bufs=4, space="PSUM"))

            # ---- constant loads (chunked so compute can start early) ----
            xt_sb = const.tile([P, CT, S], mmdt)
            # host supplies xt pre-blocked by s-block so every load is a
            # fully contiguous DRAM region (strided slices cost ~3us of
            # descriptor generation each)
            xt_rs = [
                xt_d[sb].rearrange("(ct p) s -> p ct s", p=P)
                for sb in range(S // SB)
            ]
            wt_sb = const.tile([P, CT, FPC], mmdt)
            wt_r = wt_d.rearrange("(ct p) f -> p ct f", p=P)
            bias_sb = const.tile([P, PAIRS * 3], f32)
            nc.gpsimd.dma_start(out=bias_sb, in_=bias_d.rearrange("a b -> b a"))
            # wt + xt s-block 0 in c-tile-pair chunks so the first projection
            # chains run as the data arrives; later s-blocks follow on the
            # same ring (sequential, so the critical chunks get full HBM BW).
            for c2 in range(CT // 2):
                nc.sync.dma_start(
                    out=wt_sb[:, 2 * c2 : 2 * c2 + 2, :],
                    in_=wt_r[:, 2 * c2 : 2 * c2 + 2, :],
                )
                nc.sync.dma_start(
                    out=xt_sb[:, 2 * c2 : 2 * c2 + 2, 0:SB],
                    in_=xt_rs[0][:, 2 * c2 : 2 * c2 + 2, :],
                )
            for sb in range(1, S // SB):
                nc.sync.dma_start(
                    out=xt_sb[:, :, sb * SB : (sb + 1) * SB],
                    in_=xt_rs[sb],
                )
            ident = const.tile([P, P], f32)
            make_identity(nc, ident)
            ident_mm = const.tile([P, P], mmdt)
            make_identity(nc, ident_mm)

            # Q/K storage: per pair one [128, 2, 2048] tile; partitions 0:64 =
            # head A, 64:128 = head B; free dim 0 = Q_t, 1 = K_t (both [d, s]).
            qk_sb = []
            for pair in range(PAIRS):
                qk_t = qkp.tile([P, 2, S], mmdt, name=f"qk{pair}")
                qk_sb.append(qk_t)
            # V storage: per head [128, 16, 65]: [k-tile partition, k-tile, d+1];
            # column 64 is the ones column (softmax denominator trick).
            ones_sb = const.tile([P, KT], f32)
            nc.vector.memset(ones_sb, 1.0)
            vo_sb = []
            for h in range(HPC):
                vo_t = vop.tile([P, KT, HD + 1], mmdt, name=f"vo{h}")
                vo_sb.append(vo_t)
                nc.vector.tensor_copy(vo_t[:, :, HD], ones_sb)

            vt_tiles = {}
            for pair in range(PAIRS):
                vt_tiles[pair] = vtp.tile([P, S], mmdt, name=f"vt{pair}", tag="vt")

            def emit_chain(pair, ft, sb):
                """One projection chain: psum[feat,s-block] over 8 c-tiles + copy."""
                ps = smallp.tile([P, SB], f32, name="proj_ps", tag="small")
                for ct in range(CT):
                    nc.tensor.matmul(
                        ps,
                        lhsT=wt_sb[:, ct, pair * 3 * P + ft * P : pair * 3 * P + ft * P + P],
                        rhs=xt_sb[:, ct, sb * SB : (sb + 1) * SB],
                        start=(ct == 0),
                        stop=(ct == CT - 1),
                    )
                bcol = bias_sb[:, pair * 3 + ft : pair * 3 + ft + 1]
                if ft < 2:
                    dst = qk_sb[pair][:, ft, sb * SB : (sb + 1) * SB]
                else:
                    dst = vt_tiles[pair][:, sb * SB : (sb + 1) * SB]
                nc.vector.tensor_scalar_add(dst, ps, bcol)

            def emit_vtrans(pair, kt):
                tp = smallp.tile([P, P], mmdt, name="vtp_ps", tag="small")
                nc.tensor.transpose(tp, vt_tiles[pair][:, kt * P : (kt + 1) * P], ident_mm)
                nc.vector.tensor_copy(vo_sb[2 * pair][:, kt, 0:HD], tp[:, 0:HD])
                nc.vector.tensor_copy(vo_sb[2 * pair + 1][:, kt, 0:HD], tp[:, HD:P])

            def chain_steps(pair, ft, sb):
                """A projection chain as 5 fine-grained emission steps
                (2 matmuls each + the final copy) so it can be woven into
                the ACT-gated attention loop without starving the scalar
                engine."""
                hold = {}

                def mm(ct):
                    nc.tensor.matmul(
                        hold["ps"],
                        lhsT=wt_sb[
                            :, ct, pair * 3 * P + ft * P : pair * 3 * P + ft * P + P
                        ],
                        rhs=xt_sb[:, ct, sb * SB : (sb + 1) * SB],
                        start=(ct == 0),
                        stop=(ct == CT - 1),
                    )

                def s0():
                    hold["ps"] = smallp.tile([P, SB], f32, name="proj_ps", tag="small")
                    mm(0)
                    mm(1)

                def s_mid(c):
                    mm(c)
                    mm(c + 1)

                def s4():
                    bcol = bias_sb[:, pair * 3 + ft : pair * 3 + ft + 1]
                    if ft < 2:
                        dst = qk_sb[pair][:, ft, sb * SB : (sb + 1) * SB]
                    else:
                        dst = vt_tiles[pair][:, sb * SB : (sb + 1) * SB]
                    nc.vector.tensor_scalar_add(dst, hold["ps"], bcol)

                return [s0, lambda: s_mid(2), lambda: s_mid(4), lambda: s_mid(6), s4]

            def vt_step(pair, kt):
                def s():
                    emit_vtrans(pair, kt)

                return [s]

            def qb0_weave(pair):
                """Per-kt schedule for a pair's own first q-block carrying the
                rest of its projection: K s-blocks 1-3 (due by kt=4*sb), all V
                chains + transposes (vtrans(kt) due by O(kt-ODELAY)), Q s-block
                1 (due by qb1). Entry [KT] runs after the k-loop."""
                K1 = chain_steps(pair, 1, 1)
                K2 = chain_steps(pair, 1, 2)
                K3 = chain_steps(pair, 1, 3)
                V0 = chain_steps(pair, 2, 0)
                V1 = chain_steps(pair, 2, 1)
                V2 = chain_steps(pair, 2, 2)
                V3 = chain_steps(pair, 2, 3)
                Q1 = chain_steps(pair, 0, 1)
                vt = [vt_step(pair, k)[0] for k in range(KT)]
                return [
                    [],
                    K1[0:3],
                    K1[3:5] + V0[0:2],
                    V0[2:5] + [vt[0]],
                    [vt[1], vt[2], vt[3]] + K2[0:1],
                    K2[1:4],
                    K2[4:5] + V1[0:2],
                    V1[2:5] + [vt[4]],
                    [vt[5], vt[6], vt[7]] + K3[0:1],
                    K3[1:4],
                    K3[4:5] + V2[0:2],
                    V2[2:5] + [vt[8]],
                    [vt[9], vt[10], vt[11]] + Q1[0:1],
                    Q1[1:4],
                    Q1[4:5] + V3[0:2],
                    V3[2:5] + [vt[12]],
                    [vt[13], vt[14], vt[15]],
                ]

            ODELAY = 4  # software-pipeline depth for the O matmuls

            pending_norm = {"fn": None}

            def emit_attention(pair, weaves):
                """weaves[qb] = list of emission steps to spread across that
                q-block's 16 k-iterations."""
                qk_t = qk_sb[pair]
                for qb in range(S // SB):  # q-blocks of 512 per head
                    w = weaves[qb] if qb < len(weaves) else []
                    if w and isinstance(w[0], list):
                        per_kt, post = w[:KT], (w[KT] if len(w) > KT else [])
                    else:
                        # spread flat lists over kts 3..13, keeping q-block
                        # boundaries free of extra PE work
                        per_kt = [[] for _ in range(KT)]
                        for j, step in enumerate(w):
                            per_kt[3 + (j * 11) // max(len(w), 1)].append(step)
                        post = []
                    o_ps = [
                        smallp.tile([HD + 1, SB], f32, name=f"o_ps{i}", tag="small")
                        for i in range(2)
                    ]
                    ets = []

                    def s_mm(dst, kt, half):
                        pb = half * HD
                        nc.tensor.matmul(
                            dst,
                            lhsT=qk_t[pb : pb + HD, 1, kt * P : (kt + 1) * P],
                            rhs=qk_t[pb : pb + HD, 0, qb * SB : (qb + 1) * SB],
                            start=True,
                            stop=True,
                        )

                    def o_mm(kt, half, rhs):
                        nc.tensor.matmul(
                            o_ps[half],
                            lhsT=vo_sb[2 * pair + half][:, kt, :],
                            rhs=rhs,
                            start=(kt == 0),
                            stop=(kt == KT - 1),
                        )

                    if S_PSUM_F16:
                        # fp16 score tiles spanning two k-tiles: one
                        # [128, 2048] exp per 2 iterations (halves ACT
                        # per-instruction overhead).
                        KP = KT // 2
                        OD2 = (ODELAY + 1) // 2

                        def emit_o2(kp):
                            for sub in range(2):
                                for half in range(2):
                                    o_mm(
                                        2 * kp + sub,
                                        half,
                                        ets[kp][
                                            :,
                                            sub * QB
                                            + half * SB : sub * QB
                                            + (half + 1) * SB,
                                        ],
                                    )

                        for kp in range(KP):
                            s_ps = bigp.tile(
                                [P, 2 * QB], f16, name="s_ps", tag="big"
                            )
                            for sub in range(2):
                                for half in range(2):
                                    s_mm(
                                        s_ps[
                                            :,
                                            sub * QB
                                            + half * SB : sub * QB
                                            + (half + 1) * SB,
                                        ],
                                        2 * kp + sub,
                                        half,
                                    )
                            et = etp.tile([P, 2 * QB], mmdt, name="et", tag="et")
                            nc.scalar.activation(et, s_ps, EXP, scale=0.125)
                            ets.append(et)
                            for s in per_kt[2 * kp] + per_kt[2 * kp + 1]:
                                s()
                            if kp == 1 and pending_norm["fn"] is not None:
                                pending_norm["fn"]()
                                pending_norm["fn"] = None
                            if kp == 3 and pending_norm.get("fn2") is not None:
                                pending_norm["fn2"]()
                                pending_norm["fn2"] = None
                            if kp >= OD2:
                                emit_o2(kp - OD2)
                        for s in post:
                            s()
                        for kp in range(KP - OD2, KP):
                            emit_o2(kp)
                    else:
                        def emit_o(kt):
                            for half in range(2):
                                o_mm(
                                    kt,
                                    half,
                                    ets[kt][:, half * SB : (half + 1) * SB],
                                )

                        for kt in range(KT):
                            s_ps = bigp.tile([P, QB], f32, name="s_ps", tag="big")
                            for half in range(2):  # head A / B row groups
                                s_mm(
                                    s_ps[:, half * SB : (half + 1) * SB], kt, half
                                )
                            et = etp.tile([P, QB], mmdt, name="et", tag="et")
                            nc.scalar.activation(et, s_ps, EXP, scale=0.125)
                            ets.append(et)
                            for s in per_kt[kt]:
                                s()
                            if kt == 2 and pending_norm["fn"] is not None:
                                pending_norm["fn"]()
                                pending_norm["fn"] = None
                            if kt == 6 and pending_norm.get("fn2") is not None:
                                pending_norm["fn2"]()
                                pending_norm["fn2"] = None
                            if kt >= ODELAY:
                                emit_o(kt - ODELAY)
                        for s in post:
                            s()
                        for kt in range(KT - ODELAY, KT):
                            emit_o(kt)
                    # normalize + transpose back to [q, d], then store —
                    # deferred into the next q-block's warm-up window
                    def normalize(half, qb=qb, o_ps=o_ps):
                        if True:
                            h = 2 * pair + half
                            osb = osbp.tile([HD + 1, SB], f32, name="osb", tag="osb")
                            nc.vector.tensor_copy(osb, o_ps[half])
                            ofin = ofinp.tile([P, SB // P, HD], f32, name="ofin", tag="ofin")
                            for j in range(SB // P):
                                tp2 = smallp.tile([P, HD + 1], f32, name="ot_ps", tag="small")
                                nc.tensor.transpose(
                                    tp2,
                                    osb[:, j * P : (j + 1) * P],
                                    ident[: HD + 1, : HD + 1],
                                )
                                rc = rcp.tile([P, 1], f32, name="rc", tag="rc")
                                nc.vector.reciprocal(rc, tp2[:, HD : HD + 1])
                                nc.vector.tensor_scalar_mul(ofin[:, j, :], tp2[:, 0:HD], rc)
                            nc.sync.dma_start(
                                out=out_d[h, qb * SB : (qb + 1) * SB, :].rearrange(
                                    "(j p) d -> p j d", p=P
                                ),
                                in_=ofin,
                            )
                    pending_norm["fn"] = lambda n=normalize: n(0)
                    pending_norm["fn2"] = lambda n=normalize: n(1)

            def run_steps(steps):
                for s in steps:
                    s()

            # upfront: K/Q chains for s-block 0 of pair 0 (minimum to start).
            # A few dependency-free warm-up matmuls fill the DMA-wait gaps so
            # the HAM clock gate opens before the real chains run.
            warm_sb = const.tile([P, SB], mmdt)
            nc.vector.memset(warm_sb, 0.0)
            warm_ps = smallp.tile([P, SB], f32, name="warm_ps", tag="small")

            def warm():
                nc.tensor.matmul(
                    warm_ps, lhsT=warm_sb[:, 0:P], rhs=warm_sb, start=True, stop=True
                )

            for s_k, s_q in zip(chain_steps(0, 1, 0), chain_steps(0, 0, 0)):
                warm()
                s_k()
                warm()
                s_q()
            # pair-1 projection is split between pair-0's underloaded later
            # q-blocks and pair-1's own first q-block.
            emit_attention(
                0,
                [
                    qb0_weave(0),
                    chain_steps(0, 0, 2) + chain_steps(1, 1, 0)  # Q2, K'0
                    + chain_steps(1, 0, 0),  # Q'0
                    chain_steps(0, 0, 3) + chain_steps(1, 1, 1)  # Q3, K'1
                    + chain_steps(1, 2, 0)  # V'0
                    + vt_step(1, 0) + vt_step(1, 1) + vt_step(1, 2) + vt_step(1, 3),
                    chain_steps(1, 2, 1)  # V'1
                    + vt_step(1, 4) + vt_step(1, 5) + vt_step(1, 6) + vt_step(1, 7)
                    + chain_steps(1, 1, 2)  # K'2
                    + chain_steps(1, 2, 2)  # V'2
                    + vt_step(1, 8) + vt_step(1, 9) + vt_step(1, 10) + vt_step(1, 11),
                ],
            )
            emit_attention(
                1,
                [
                    chain_steps(1, 1, 3)  # K'3
                    + chain_steps(1, 0, 1)  # Q'1
                    + chain_steps(1, 2, 3)  # V'3
                    + vt_step(1, 12) + vt_step(1, 13) + vt_step(1, 14) + vt_step(1, 15),
                    chain_steps(1, 0, 2),
                    chain_steps(1, 0, 3),
                    [],
                ],
            )
            if pending_norm["fn"] is not None:
                pending_norm["fn"]()
                pending_norm["fn"] = None
            if pending_norm.get("fn2") is not None:
                pending_norm["fn2"]()
                pending_norm["fn2"] = None

    nc.finalize()
    return nc


def _get_program():
    if "nc" not in _CACHE:
        _CACHE["nc"] = _build_program()
    return _CACHE["nc"]


def _make_in_maps(x, w_qkv, b_qkv):
    in_maps = []
    for core in range(N_CORES):
        b, g = core // 4, core % 4
        order = []
        for pair in range(PAIRS):
            hA = 4 * g + 2 * pair
            for off in (0, HD, 2 * HD):  # Q, K, V row offsets inside a head
                for h in (hA, hA + 1):
                    order.extend(range(h * 3 * HD + off, h * 3 * HD + off + HD))
        order = np.asarray(order)
        if MM_DTYPE == "bf16":
            import ml_dtypes

            cvt = lambda a: np.ascontiguousarray(a.astype(ml_dtypes.bfloat16))
        elif MM_DTYPE == "f16":
            cvt = lambda a: np.ascontiguousarray(a.astype(np.float16))
        else:
            cvt = np.ascontiguousarray
        in_maps.append(
            {
                "xt": cvt(
                    x[b].T.reshape(D_MODEL, S // SB, SB).transpose(1, 0, 2)
                ),
                "wt": cvt(w_qkv[order].T),
                "bias": np.ascontiguousarray(b_qkv[order].reshape(PAIRS * 3, P)),
            }
        )
    return in_maps


def _install_ntff_hook():
    """Provide antenv.axon_hooks (absent in this image) so trace=True works."""
    import contextlib
    import ctypes
    import types

    try:
        from antenv.axon_hooks import get_axon_ntff_profile_hook  # noqa: F401

        return
    except ImportError:
        pass
    import antenv

    mod = types.ModuleType("antenv.axon_hooks")
    mod._hook = None

    def set_axon_ntff_profile_hook(h):
        mod._hook = h

    def get_axon_ntff_profile_hook():
        return mod._hook

    mod.set_axon_ntff_profile_hook = set_axon_ntff_profile_hook
    mod.get_axon_ntff_profile_hook = get_axon_ntff_profile_hook
    sys.modules["antenv.axon_hooks"] = mod
    antenv.axon_hooks = mod

    so_path = "/opt/axon/libaxon_pjrt.so"
    if not os.path.exists(so_path):
        return
    lib = ctypes.CDLL(so_path)
    if not hasattr(lib, "axon_start_nrt_profile"):
        return
    lib.axon_start_nrt_profile.argtypes = [
        ctypes.POINTER(ctypes.c_int64),
        ctypes.c_size_t,
    ]
    lib.axon_start_nrt_profile.restype = ctypes.c_int64
    lib.axon_stop_nrt_profile.argtypes = [ctypes.c_char_p]
    lib.axon_stop_nrt_profile.restype = ctypes.c_int64

    @contextlib.contextmanager
    def _hook(output_dir, device_ids):
        import jax

        jax.devices()
        if device_ids:
            ids = (ctypes.c_int64 * len(device_ids))(*device_ids)
            rc = lib.axon_start_nrt_profile(ids, len(device_ids))
        else:
            rc = lib.axon_start_nrt_profile(None, 0)
        if rc != 0:
            raise RuntimeError(f"axon_start_nrt_profile rc={rc}")
        try:
            yield
        finally:
            n = lib.axon_stop_nrt_profile(str(output_dir).encode())
            print(f"profile: {n} file(s) written to {output_dir}")

    set_axon_ntff_profile_hook(_hook)


def kernel(x, w_qkv, b_qkv, trace=False):
    from concourse.bass_utils import run_bass_kernel_spmd

    if trace:
        _install_ntff_hook()

    x = np.ascontiguousarray(np.asarray(x, dtype=np.float32))
    w_qkv = np.ascontiguousarray(np.asarray(w_qkv, dtype=np.float32))
    b_qkv = np.ascontiguousarray(np.asarray(b_qkv, dtype=np.float32))

    nc = _get_program()
    in_maps = _make_in_maps(x, w_qkv, b_qkv)
    try:
        res = run_bass_kernel_spmd(nc, in_maps, list(range(N_CORES)), trace=trace)
    except Exception:
        # transient device-load failures have been observed under axon;
        # one retry after a cooldown clears them
        import time

        time.sleep(10)
        res = run_bass_kernel_spmd(nc, in_maps, list(range(N_CORES)), trace=trace)
    _CACHE["last_result"] = res

    out = np.empty((B, S, D_MODEL), dtype=np.float32)
    for core in range(N_CORES):
        b, g = core // 4, core % 4
        out[b].reshape(NUM_HEADS, S, HD)[4 * g : 4 * g + 4] = res.results[core]["out"]
    return out
